# revision 5
# baseline (speedup 1.0000x reference)
"""GRU decoder (nn_Decoder) on 8 TRN2 NeuronCores — v5.

vs v1: HWDGE bounce DMAs on the exchange path, gi folded into the gh PSUM
accumulation (no extract/copy/add chain), sigmoid/exp read PSUM directly,
DVE block-transposes replace PE transpose + copy, fp8 DoubleRow output
projection (wout*16 in e4m3, exp(x/16) on the way out, fused row-sums via
activation accum_out), projection spread as 2 vocab tiles per step, and
denominator AllReduces batched 8 chunks at a time so they stay off the
per-step AllGather queue.
"""

import numpy as np
import ml_dtypes

VOCAB = 32000
EMB = 512
HID = 1024
B = 32
SEQ = 64
PAD_ID = 0
SOS_ID = 1
EOS_ID = 2
NC = 8
HSL = HID // NC          # 128 h columns per core
GS = 3 * HSL             # 384 gate rows per core
VS = VOCAB // NC         # 4000 vocab per core
ROWS = SEQ * B           # 2048
MCH = ROWS // 128        # 16 m-chunks (4 steps each)
VT = 8                   # vocab tiles per m-chunk (4000 = 8*500)
VTW = VS // VT           # 500
WS = 16.0                # fp8 weight scale for w_out

BF16 = ml_dtypes.bfloat16
FP8 = ml_dtypes.float8_e4m3

_CACHE = {}


def _build():
    import concourse.bass as bass
    import concourse.mybir as mybir
    import concourse.tile as tile
    from concourse import bacc
    from concourse.masks import make_identity

    f32 = mybir.dt.float32
    bf16 = mybir.dt.bfloat16
    fp8 = mybir.dt.float8e4
    AF = mybir.ActivationFunctionType
    ALU = mybir.AluOpType
    AX = mybir.AxisListType
    PM = mybir.MatmulPerfMode

    nc = bacc.Bacc(None, target_bir_lowering=False, num_devices=NC)

    eT_d = nc.dram_tensor("eT", [EMB, ROWS], bf16, kind="ExternalInput")
    wihT_d = nc.dram_tensor("wihT", [EMB, GS], bf16, kind="ExternalInput")
    whhT_d = nc.dram_tensor("whhT", [HID, GS], bf16, kind="ExternalInput")
    wout8_d = nc.dram_tensor("wout8", [128, HID // 256, 2, VS], fp8, kind="ExternalInput")
    hT0_d = nc.dram_tensor("hT0", [HID, B], bf16, kind="ExternalInput")
    h0own_d = nc.dram_tensor("h0own", [B, HSL], bf16, kind="ExternalInput")
    done_d = nc.dram_tensor("doneS", [B, SEQ], f32, kind="ExternalInput")
    live_d = nc.dram_tensor("livePB", [128, MCH], f32, kind="ExternalInput")
    pad_d = nc.dram_tensor("padPB", [128, MCH], f32, kind="ExternalInput")
    out_d = nc.dram_tensor("out", [ROWS, VS], f32, kind="ExternalOutput")

    with tile.TileContext(nc) as tc:
        with (
            tc.tile_pool(name="wts", bufs=1) as wts,
            tc.tile_pool(name="state", bufs=1) as state,
            tc.tile_pool(name="hown", bufs=2) as hown_pool,
            tc.tile_pool(name="gtmp", bufs=3) as gtmp,
            tc.tile_pool(name="h8p", bufs=3) as h8p,
            tc.tile_pool(name="pgh", bufs=2, space="PSUM") as pgh_pool,
            tc.tile_pool(name="pgin", bufs=2, space="PSUM") as pgin_pool,
            tc.tile_pool(name="ppb", bufs=3, space="PSUM") as ppb_pool,
            tc.tile_pool(name="dram", bufs=4, space="DRAM") as dram,
            tc.tile_pool(name="dram2", bufs=2, space="DRAM") as dram2,
        ):
            whhT = wts.tile([128, HID // 128, GS], bf16)
            wout8 = wts.tile([128, HID // 256, 2, VS], fp8)      # 4.1 MB
            gi = state.tile([128, MCH, GS], bf16)
            hT = state.tile([128, SEQ + 1, NC, B], bf16)
            doneS = state.tile([B, SEQ], f32)
            livePB = state.tile([128, MCH], f32)
            padPB = state.tile([128, MCH], f32)
            ident = state.tile([128, 128], bf16)
            sums = state.tile([128, MCH], f32)
            denoms = state.tile([128, MCH], f32)

            make_identity(nc, ident[:])

            nc.scalar.dma_start(whhT[:], whhT_d.rearrange("(c p) n -> p c n", p=128))
            nc.scalar.dma_start(wout8[:], wout8_d[:])
            nc.sync.dma_start(hT[:, 0, :, :], hT0_d.rearrange("(c p) n -> p c n", p=128))
            nc.sync.dma_start(doneS[:], done_d[:])
            nc.sync.dma_start(livePB[:], live_d[:])
            nc.sync.dma_start(padPB[:], pad_d[:])

            h_own_init = hown_pool.tile([B, HSL], bf16, name="h_own")
            nc.sync.dma_start(h_own_init[:], h0own_d[:])
            h_own = h_own_init

            # ---- phase 0: gi[m] = (E @ w_ih_slice.T)[128 rows] ----
            with nc.named_scope("ph0"), tc.tile_pool(name="ph0", bufs=1) as ph0, \
                 tc.tile_pool(name="pgi0", bufs=1, space="PSUM") as pgi0_pool:
                eT = ph0.tile([128, EMB // 128, ROWS], bf16)
                wihT = ph0.tile([128, EMB // 128, GS], bf16)
                nc.scalar.dma_start(eT[:], eT_d.rearrange("(c p) n -> p c n", p=128))
                nc.scalar.dma_start(wihT[:], wihT_d.rearrange("(c p) n -> p c n", p=128))
                for m in range(MCH):
                    pgi = pgi0_pool.tile([128, GS], f32, name="pgi0")
                    for k in range(EMB // 128):
                        nc.tensor.matmul(
                            pgi[:],
                            eT[:, k, bass.ts(m, 128)],
                            wihT[:, k, :],
                            start=(k == 0),
                            stop=(k == EMB // 128 - 1),
                        )
                    nc.vector.tensor_copy(gi[:, m, :], pgi[:])

            expp_ctx = tc.tile_pool(name="expp", bufs=10)
            outp_ctx = tc.tile_pool(name="outp", bufs=2)
            expp = expp_ctx.__enter__()
            outp = outp_ctx.__enter__()

            def exchange_ag(src_sb, t):
                # send h untransposed [B, HSL]; the receive DMA transposes in
                # hardware (xbar) straight into hT[:, t+1, :, :] = [128, (k b)].
                agin = dram.tile([B, HSL], bf16, name="agin")
                agout = dram2.tile([NC * B, HSL], bf16, name="agout")
                nc.sync.dma_start(agin[:], src_sb[:])
                nc.gpsimd.collective_compute(
                    "AllGather",
                    mybir.AluOpType.bypass,
                    replica_groups=[list(range(NC))],
                    ins=[agin.opt()],
                    outs=[agout.opt()],
                )
                nc.sync.dma_start(
                    hT[:, t + 1, :, :].rearrange("p k b -> p (k b)"),
                    agout[:],
                    transpose=True,
                )

            # ---- phase A single step ----
            def step(t):
                nonlocal h_own
                with nc.named_scope(f"stA{t:02d}"):
                    _step_body(t)

            def _step_body(t):
                nonlocal h_own
                m, po = t // 4, (t % 4) * B
                # gin extract early (independent of this step's gh)
                pgin = pgin_pool.tile([B, HSL], f32, name="pgin")
                nc.tensor.matmul(pgin[:], ident[:, po:po + B], gi[:, m, 2 * HSL:],
                                 start=True, stop=True)
                pgh = pgh_pool.tile([B, GS], f32, name="pgh")
                for k in range(NC):
                    nc.tensor.matmul(
                        pgh[:],
                        hT[:, t, k, :],
                        whhT[:, k, :],
                        start=(k == 0),
                        stop=False,
                        skip_group_check=True,
                    )
                # fold gi r,z into pgh[:, :2*HSL] via PSUM accumulation
                nc.tensor.matmul(pgh[:, :2 * HSL], ident[:, po:po + B],
                                 gi[:, m, :2 * HSL], start=False, stop=True,
                                 skip_group_check=True)
                rz = gtmp.tile([B, 2 * HSL], f32, name="rz")
                nc.scalar.activation(rz[:], pgh[:, :2 * HSL], AF.Sigmoid)
                t1 = gtmp.tile([B, HSL], f32, name="t1")
                nc.vector.tensor_tensor(t1[:], rz[:, :HSL], pgh[:, 2 * HSL:], ALU.mult)
                nc.vector.tensor_tensor(t1[:], t1[:], pgin[:], ALU.add)
                zp = gtmp.tile([B, HSL], f32, name="zp")
                nc.vector.tensor_scalar(zp[:], rz[:, HSL:], doneS[:, t:t + 1], None, ALU.max)
                n_t = gtmp.tile([B, HSL], f32, name="n_t")
                nc.scalar.activation(n_t[:], t1[:], AF.Tanh)
                d_t = gtmp.tile([B, HSL], f32, name="d_t")
                nc.vector.tensor_tensor(d_t[:], n_t[:], h_own[:], ALU.subtract)
                nc.vector.tensor_tensor(d_t[:], zp[:], d_t[:], ALU.mult)
                h_new = hown_pool.tile([B, HSL], bf16, name="h_own")
                nc.vector.tensor_tensor(h_new[:], n_t[:], d_t[:], ALU.subtract)
                h_own = h_new
                with nc.named_scope(f"ag{t:02d}"):
                    exchange_ag(h_new, t)

            # ---- phase B pieces ----
            def pb_cast(m):
                """fp8 copy of hT chunk m (steps 4m+1..4m+4) for DoubleRow."""
                h8 = h8p.tile([128, NC, 4, B], fp8, name="h8")
                nc.vector.tensor_copy(
                    h8[:], hT[:, 4 * m + 1:4 * m + 5, :, :].rearrange("p t k b -> p k t b"))
                return h8

            def pb_vtiles(m, h8, expb, v0, nv):
                with nc.named_scope(f"pb{m:02d}_{v0}"):
                    for v in range(v0, v0 + nv):
                        ppb = ppb_pool.tile([128, VTW], f32, name="ppb")
                        for u in range(HID // 256):
                            nc.tensor.matmul(
                                ppb[:],
                                h8[:, 2 * u:2 * u + 2, :, :],
                                wout8[:, u, :, bass.ts(v, VTW)],
                                start=(u == 0),
                                stop=(u == HID // 256 - 1),
                                perf_mode=PM.DoubleRow,
                            )
                        s_v = gtmp.tile([128, 1], f32, name="s_v")
                        nc.scalar.activation(expb[:, bass.ts(v, VTW)], ppb[:],
                                             AF.Exp, scale=1.0 / WS, accum_out=s_v[:])
                        if v == 0:
                            nc.vector.tensor_copy(sums[:, m:m + 1], s_v[:])
                        else:
                            nc.vector.tensor_tensor(sums[:, m:m + 1], sums[:, m:m + 1],
                                                    s_v[:], ALU.add)

            def pb_allreduce(m_lo, m_hi):
                # AllGather (4.6us floor) + local reduce instead of AllReduce
                # (9.7us floor): each core only needs the 8 partial sums.
                with nc.named_scope(f"ar{m_hi:02d}"):
                    nb = m_hi - m_lo + 1
                    arin = dram.tile([128, nb], f32, name="arin")
                    arout = dram2.tile([NC * 128, nb], f32, name="arout")
                    nc.sync.dma_start(arin[:], sums[:, m_lo:m_hi + 1])
                    nc.gpsimd.collective_compute(
                        "AllGather",
                        mybir.AluOpType.bypass,
                        replica_groups=[list(range(NC))],
                        ins=[arin.opt()],
                        outs=[arout.opt()],
                    )
                    gsums = gtmp.tile([128, nb, NC], f32, name="gsums")
                    nc.sync.dma_start(gsums[:], arout.rearrange("(c p) n -> p n c", p=128))
                    nc.vector.reduce_sum(denoms[:, m_lo:m_hi + 1], gsums[:], AX.X)

            def pb_norm(m, expb):
                with nc.named_scope(f"pbn{m:02d}"):
                    inv = gtmp.tile([128, 1], f32, name="inv")
                    nc.vector.reciprocal(inv[:], denoms[:, m:m + 1])
                    sc = gtmp.tile([128, 1], f32, name="sc")
                    nc.vector.tensor_tensor(sc[:], livePB[:, m:m + 1], inv[:], ALU.mult)
                    ouf = outp.tile([128, VS], f32, name="ouf")
                    nc.vector.tensor_scalar(ouf[:], expb[:], sc[:], None, ALU.mult)
                    nc.vector.tensor_tensor(ouf[:, 0:1], ouf[:, 0:1], padPB[:, m:m + 1], ALU.add)
                    nc.scalar.dma_start(out_d[bass.ts(m, 128), :], ouf[:])

            # ---- main interleaved schedule ----
            # chunk m's hT slots land after step 4m+3; process its 8 v-tiles
            # as 2 per step during steps 4m+4 .. 4m+7.
            live = {}     # m -> (h8, expb)
            pending = []  # chunks awaiting denominators
            normq = []    # denominator-ready chunks, one normalized per step
            for t in range(SEQ):
                step(t)
                if t >= 4 and t % 4 == 0:
                    m = t // 4 - 1
                    h8 = pb_cast(m)
                    expb = expp.tile([128, VS], bf16, name="expb")
                    live[m] = (h8, expb)
                if t >= 4:
                    m = (t - 4) // 4
                    h8, expb = live[m]
                    pb_vtiles(m, h8, expb, 2 * (t % 4), 2)
                    if t % 4 == 3:
                        pending.append((m, expb))
                        del live[m]
                        if m == 7:
                            pb_allreduce(0, 7)
                            normq = pending
                            pending = []
                        elif m == 11:
                            pb_allreduce(8, 11)
                            normq = normq + pending
                            pending = []
                if normq:
                    pb_norm(*normq.pop(0))
            # tail: chunk 15
            m = MCH - 1
            h8 = pb_cast(m)
            expb = expp.tile([128, VS], bf16, name="expb")
            pb_vtiles(m, h8, expb, 0, VT)
            pending.append((m, expb))
            pb_allreduce(12, 15)
            for pm, pe in normq + pending:
                pb_norm(pm, pe)

            outp_ctx.__exit__(None, None, None)
            expp_ctx.__exit__(None, None, None)

    nc.compile()
    return nc


def _host_prep(hidden, target, lenseq, emb, w_ih, w_hh, b_ih, b_hh, w_out, b_out):
    assert not np.asarray(b_ih).any() and not np.asarray(b_hh).any() and not np.asarray(b_out).any(), (
        "nonzero biases not supported by this kernel build"
    )
    target = np.asarray(target)
    X = np.empty((SEQ, B), dtype=np.int64)
    X[0] = SOS_ID
    X[1:] = target[:SEQ - 1]
    done = ((X == EOS_ID) | (X == PAD_ID)).astype(np.float32)  # [SEQ, B]
    emb = np.asarray(emb, dtype=np.float32)
    E = emb[X.reshape(-1)]                                     # [2048, 512]
    eT = np.ascontiguousarray(E.T).astype(BF16)                # [512, 2048]
    h0 = np.asarray(hidden, dtype=np.float32)[0]               # [32, 1024]
    hT0 = np.ascontiguousarray(h0.T).astype(BF16)              # [1024, 32]
    done_s = np.ascontiguousarray(done.T)                      # [B, SEQ]
    done_row = done.reshape(ROWS)                              # [2048]
    done_pb = np.ascontiguousarray(done_row.reshape(MCH, 128).T)  # [128, 16]
    live_pb = np.ascontiguousarray(1.0 - done_pb)
    w_ih = np.asarray(w_ih, dtype=np.float32)
    w_hh = np.asarray(w_hh, dtype=np.float32)
    w_out = np.asarray(w_out, dtype=np.float32)

    in_maps = []
    for c in range(NC):
        rows = np.r_[c * HSL:(c + 1) * HSL,
                     HID + c * HSL:HID + (c + 1) * HSL,
                     2 * HID + c * HSL:2 * HID + (c + 1) * HSL]
        wihT = np.ascontiguousarray(w_ih[rows].T).astype(BF16)     # [512, 384]
        whhT = np.ascontiguousarray(w_hh[rows].T).astype(BF16)     # [1024, 384]
        woutT = np.ascontiguousarray(w_out[c * VS:(c + 1) * VS].T)  # [1024, 4000] f32
        # fp8 DoubleRow pair layout: wout8[p, u, j, n] = woutT[256u+128j+p, n]*WS
        w4 = (woutT * WS).reshape(HID // 256, 2, 128, VS)          # [u, j, p, n]
        wout8 = np.ascontiguousarray(w4.transpose(2, 0, 1, 3)).astype(FP8)  # [128,u,j,n]
        h0own = np.ascontiguousarray(h0[:, c * HSL:(c + 1) * HSL]).astype(BF16)
        pad_pb = done_pb if c == 0 else np.zeros_like(done_pb)
        in_maps.append({
            "eT": eT, "wihT": wihT, "whhT": whhT, "wout8": wout8,
            "hT0": hT0, "h0own": h0own, "doneS": done_s,
            "livePB": live_pb, "padPB": np.ascontiguousarray(pad_pb),
        })
    return in_maps


def kernel(hidden, target, lenseq, emb, w_ih, w_hh, b_ih, b_hh, w_out, b_out):
    from concourse.bass_utils import run_bass_kernel_spmd

    in_maps = _host_prep(hidden, target, lenseq, emb, w_ih, w_hh, b_ih, b_hh,
                         w_out, b_out)
    if "nc" not in _CACHE:
        _CACHE["nc"] = _build()
    res = run_bass_kernel_spmd(_CACHE["nc"], in_maps, core_ids=list(range(NC)))
    outs = [r["out"] for r in res.results]                     # each [2048, 4000]
    full = np.concatenate(outs, axis=1).reshape(SEQ, B, VOCAB)
    return full[:int(lenseq)]


# revision 6
# speedup vs baseline: 2.7941x; 2.7941x over previous
"""GRU decoder (nn_Decoder) on 8 TRN2 NeuronCores — v5.

vs v1: HWDGE bounce DMAs on the exchange path, gi folded into the gh PSUM
accumulation (no extract/copy/add chain), sigmoid/exp read PSUM directly,
DVE block-transposes replace PE transpose + copy, fp8 DoubleRow output
projection (wout*16 in e4m3, exp(x/16) on the way out, fused row-sums via
activation accum_out), projection spread as 2 vocab tiles per step, and
denominator AllReduces batched 8 chunks at a time so they stay off the
per-step AllGather queue.
"""

import numpy as np
import ml_dtypes

VOCAB = 32000
EMB = 512
HID = 1024
B = 32
SEQ = 64
PAD_ID = 0
SOS_ID = 1
EOS_ID = 2
NC = 8
HSL = HID // NC          # 128 h columns per core
GS = 3 * HSL             # 384 gate rows per core
VS = VOCAB // NC         # 4000 vocab per core
ROWS = SEQ * B           # 2048
MCH = ROWS // 128        # 16 m-chunks (4 steps each)
VT = 8                   # vocab tiles per m-chunk (4000 = 8*500)
VTW = VS // VT           # 500
WS = 16.0                # fp8 weight scale for w_out

BF16 = ml_dtypes.bfloat16
FP8 = ml_dtypes.float8_e4m3

_CACHE = {}


def _build():
    import concourse.bass as bass
    import concourse.mybir as mybir
    import concourse.tile as tile
    from concourse import bacc
    from concourse.masks import make_identity

    f32 = mybir.dt.float32
    bf16 = mybir.dt.bfloat16
    fp8 = mybir.dt.float8e4
    AF = mybir.ActivationFunctionType
    ALU = mybir.AluOpType
    AX = mybir.AxisListType
    PM = mybir.MatmulPerfMode

    nc = bacc.Bacc(None, target_bir_lowering=False, num_devices=NC)

    eT_d = nc.dram_tensor("eT", [EMB, ROWS], bf16, kind="ExternalInput")
    wihT_d = nc.dram_tensor("wihT", [EMB, GS], bf16, kind="ExternalInput")
    whhT_d = nc.dram_tensor("whhT", [HID, GS], bf16, kind="ExternalInput")
    wout8_d = nc.dram_tensor("wout8", [128, HID // 256, 2, VS], fp8, kind="ExternalInput")
    hT0_d = nc.dram_tensor("hT0", [HID, B], bf16, kind="ExternalInput")
    h0own_d = nc.dram_tensor("h0own", [B, HSL], bf16, kind="ExternalInput")
    done_d = nc.dram_tensor("doneS", [B, SEQ], f32, kind="ExternalInput")
    live_d = nc.dram_tensor("livePB", [128, MCH], f32, kind="ExternalInput")
    pad_d = nc.dram_tensor("padPB", [128, MCH], f32, kind="ExternalInput")
    out_d = nc.dram_tensor("out", [ROWS, VS], f32, kind="ExternalOutput")

    with tile.TileContext(nc) as tc:
        with (
            tc.tile_pool(name="wts", bufs=1) as wts,
            tc.tile_pool(name="state", bufs=1) as state,
            tc.tile_pool(name="hown", bufs=2) as hown_pool,
            tc.tile_pool(name="gtmp", bufs=5) as gtmp,
            tc.tile_pool(name="h8p", bufs=3) as h8p,
            tc.tile_pool(name="pgh", bufs=2, space="PSUM") as pgh_pool,
            tc.tile_pool(name="pgin", bufs=2, space="PSUM") as pgin_pool,
            tc.tile_pool(name="ppb", bufs=3, space="PSUM") as ppb_pool,
            tc.tile_pool(name="dram", bufs=6, space="DRAM") as dram,
            tc.tile_pool(name="dram2", bufs=4, space="DRAM") as dram2,
        ):
            whhT = wts.tile([128, HID // 128, GS], bf16)
            wout8 = wts.tile([128, HID // 256, 2, VS], fp8)      # 4.1 MB
            gi = state.tile([128, MCH, GS], bf16)
            hT = state.tile([128, SEQ + 1, NC, B], bf16)
            doneS = state.tile([B, SEQ], f32)
            livePB = state.tile([128, MCH], f32)
            padPB = state.tile([128, MCH], f32)
            ident = state.tile([128, 128], bf16)
            sums = state.tile([128, MCH], f32)
            denoms = state.tile([128, MCH], f32)

            make_identity(nc, ident[:])

            nc.scalar.dma_start(whhT[:], whhT_d.rearrange("(c p) n -> p c n", p=128))
            nc.scalar.dma_start(wout8[:], wout8_d[:])
            nc.sync.dma_start(hT[:, 0, :, :], hT0_d.rearrange("(c p) n -> p c n", p=128))
            nc.sync.dma_start(doneS[:], done_d[:])
            nc.sync.dma_start(livePB[:], live_d[:])
            nc.sync.dma_start(padPB[:], pad_d[:])

            h_own_init = hown_pool.tile([B, HSL], bf16, name="h_own")
            nc.sync.dma_start(h_own_init[:], h0own_d[:])
            h_own = h_own_init

            # ---- phase 0: gi[m] = (E @ w_ih_slice.T)[128 rows] ----
            with nc.named_scope("ph0"), tc.tile_pool(name="ph0", bufs=1) as ph0, \
                 tc.tile_pool(name="pgi0", bufs=1, space="PSUM") as pgi0_pool:
                eT = ph0.tile([128, EMB // 128, ROWS], bf16)
                wihT = ph0.tile([128, EMB // 128, GS], bf16)
                nc.scalar.dma_start(eT[:], eT_d.rearrange("(c p) n -> p c n", p=128))
                nc.scalar.dma_start(wihT[:], wihT_d.rearrange("(c p) n -> p c n", p=128))
                for m in range(MCH):
                    pgi = pgi0_pool.tile([128, GS], f32, name="pgi0")
                    for k in range(EMB // 128):
                        nc.tensor.matmul(
                            pgi[:],
                            eT[:, k, bass.ts(m, 128)],
                            wihT[:, k, :],
                            start=(k == 0),
                            stop=(k == EMB // 128 - 1),
                        )
                    nc.vector.tensor_copy(gi[:, m, :], pgi[:])

            expp_ctx = tc.tile_pool(name="expp", bufs=9)
            outp_ctx = tc.tile_pool(name="outp", bufs=2)
            expp = expp_ctx.__enter__()
            outp = outp_ctx.__enter__()

            def exchange_ag(src_sb, t):
                # send h untransposed [B, HSL]; the receive DMA transposes in
                # hardware (xbar) straight into hT[:, t+1, :, :] = [128, (k b)].
                agin = dram.tile([B, HSL], bf16, name="agin")
                agout = dram2.tile([NC * B, HSL], bf16, name="agout")
                nc.sync.dma_start(agin[:], src_sb[:])
                nc.gpsimd.collective_compute(
                    "AllGather",
                    mybir.AluOpType.bypass,
                    replica_groups=[list(range(NC))],
                    ins=[agin.opt()],
                    outs=[agout.opt()],
                )
                nc.sync.dma_start(
                    hT[:, t + 1, :, :].rearrange("p k b -> p (k b)"),
                    agout[:],
                    transpose=True,
                )

            # ---- phase A single step ----
            def step(t):
                nonlocal h_own
                with nc.named_scope(f"stA{t:02d}"):
                    _step_body(t)

            def _step_body(t):
                nonlocal h_own
                m, po = t // 4, (t % 4) * B
                # gin extract early (independent of this step's gh)
                pgin = pgin_pool.tile([B, HSL], f32, name="pgin")
                nc.tensor.matmul(pgin[:], ident[:, po:po + B], gi[:, m, 2 * HSL:],
                                 start=True, stop=True)
                pgh = pgh_pool.tile([B, GS], f32, name="pgh")
                for k in range(NC):
                    nc.tensor.matmul(
                        pgh[:],
                        hT[:, t, k, :],
                        whhT[:, k, :],
                        start=(k == 0),
                        stop=False,
                        skip_group_check=True,
                    )
                # fold gi r,z into pgh[:, :2*HSL] via PSUM accumulation
                nc.tensor.matmul(pgh[:, :2 * HSL], ident[:, po:po + B],
                                 gi[:, m, :2 * HSL], start=False, stop=True,
                                 skip_group_check=True)
                rz = gtmp.tile([B, 2 * HSL], f32, name="rz")
                nc.scalar.activation(rz[:], pgh[:, :2 * HSL], AF.Sigmoid)
                t1 = gtmp.tile([B, HSL], f32, name="t1")
                nc.vector.tensor_tensor(t1[:], rz[:, :HSL], pgh[:, 2 * HSL:], ALU.mult)
                nc.vector.tensor_tensor(t1[:], t1[:], pgin[:], ALU.add)
                zp = gtmp.tile([B, HSL], f32, name="zp")
                nc.vector.tensor_scalar(zp[:], rz[:, HSL:], doneS[:, t:t + 1], None, ALU.max)
                n_t = gtmp.tile([B, HSL], f32, name="n_t")
                nc.scalar.activation(n_t[:], t1[:], AF.Tanh)
                d_t = gtmp.tile([B, HSL], f32, name="d_t")
                nc.vector.tensor_tensor(d_t[:], n_t[:], h_own[:], ALU.subtract)
                nc.vector.tensor_tensor(d_t[:], zp[:], d_t[:], ALU.mult)
                h_new = hown_pool.tile([B, HSL], bf16, name="h_own")
                nc.vector.tensor_tensor(h_new[:], n_t[:], d_t[:], ALU.subtract)
                h_own = h_new
                with nc.named_scope(f"ag{t:02d}"):
                    exchange_ag(h_new, t)

            # ---- phase B pieces ----
            def pb_cast(m):
                """fp8 copy of hT chunk m (steps 4m+1..4m+4) for DoubleRow."""
                h8 = h8p.tile([128, NC, 4, B], fp8, name="h8")
                nc.vector.tensor_copy(
                    h8[:], hT[:, 4 * m + 1:4 * m + 5, :, :].rearrange("p t k b -> p k t b"))
                return h8

            def pb_vtiles(m, h8, expb, v0, nv):
                with nc.named_scope(f"pb{m:02d}_{v0}"):
                    for v in range(v0, v0 + nv):
                        ppb = ppb_pool.tile([128, VTW], f32, name="ppb")
                        for u in range(HID // 256):
                            nc.tensor.matmul(
                                ppb[:],
                                h8[:, 2 * u:2 * u + 2, :, :],
                                wout8[:, u, :, bass.ts(v, VTW)],
                                start=(u == 0),
                                stop=(u == HID // 256 - 1),
                                perf_mode=PM.DoubleRow,
                            )
                        s_v = gtmp.tile([128, 1], f32, name="s_v")
                        nc.scalar.activation(expb[:, bass.ts(v, VTW)], ppb[:],
                                             AF.Exp, scale=1.0 / WS, accum_out=s_v[:])
                        if v == 0:
                            nc.vector.tensor_copy(sums[:, m:m + 1], s_v[:])
                        else:
                            nc.vector.tensor_tensor(sums[:, m:m + 1], sums[:, m:m + 1],
                                                    s_v[:], ALU.add)

            def pb_allreduce(m_lo, m_hi):
                # AllGather (4.6us floor) + local reduce instead of AllReduce
                # (9.7us floor): each core only needs the 8 partial sums.
                with nc.named_scope(f"ar{m_hi:02d}"):
                    nb = m_hi - m_lo + 1
                    arin = dram.tile([128, nb], f32, name="arin")
                    arout = dram2.tile([NC * 128, nb], f32, name="arout")
                    nc.sync.dma_start(arin[:], sums[:, m_lo:m_hi + 1])
                    nc.gpsimd.collective_compute(
                        "AllGather",
                        mybir.AluOpType.bypass,
                        replica_groups=[list(range(NC))],
                        ins=[arin.opt()],
                        outs=[arout.opt()],
                    )
                    gsums = gtmp.tile([128, nb, NC], f32, name="gsums")
                    nc.sync.dma_start(gsums[:], arout.rearrange("(c p) n -> p n c", p=128))
                    nc.vector.reduce_sum(denoms[:, m_lo:m_hi + 1], gsums[:], AX.X)

            def pb_norm(m, expb):
                with nc.named_scope(f"pbn{m:02d}"):
                    inv = gtmp.tile([128, 1], f32, name="inv")
                    nc.vector.reciprocal(inv[:], denoms[:, m:m + 1])
                    sc = gtmp.tile([128, 1], f32, name="sc")
                    nc.vector.tensor_tensor(sc[:], livePB[:, m:m + 1], inv[:], ALU.mult)
                    ouf = outp.tile([128, VS], f32, name="ouf")
                    nc.vector.tensor_scalar(ouf[:], expb[:], sc[:], None, ALU.mult)
                    nc.vector.tensor_tensor(ouf[:, 0:1], ouf[:, 0:1], padPB[:, m:m + 1], ALU.add)
                    nc.scalar.dma_start(out_d[bass.ts(m, 128), :], ouf[:])

            # ---- main interleaved schedule ----
            # chunk m's hT slots land after step 4m+3; process its 8 v-tiles
            # as 2 per step during steps 4m+4 .. 4m+7.
            live = {}     # m -> (h8, expb)
            pending = []  # chunks awaiting denominators
            normq = []    # denominator-ready chunks, one normalized per step
            for t in range(SEQ):
                step(t)
                if t >= 4 and t % 4 == 0:
                    m = t // 4 - 1
                    h8 = pb_cast(m)
                    expb = expp.tile([128, VS], bf16, name="expb")
                    live[m] = (h8, expb)
                if t >= 4:
                    m = (t - 4) // 4
                    h8, expb = live[m]
                    pb_vtiles(m, h8, expb, 2 * (t % 4), 2)
                    if t % 4 == 3:
                        pending.append((m, expb))
                        del live[m]
                        if m == 7:
                            pb_allreduce(0, 7)
                            normq = pending
                            pending = []
                        elif m == 11:
                            pb_allreduce(8, 11)
                            normq = normq + pending
                            pending = []
                if normq:
                    pb_norm(*normq.pop(0))
            # tail: chunk 15
            m = MCH - 1
            h8 = pb_cast(m)
            expb = expp.tile([128, VS], bf16, name="expb")
            pb_vtiles(m, h8, expb, 0, VT)
            pending.append((m, expb))
            pb_allreduce(12, 15)
            for pm, pe in normq + pending:
                pb_norm(pm, pe)

            outp_ctx.__exit__(None, None, None)
            expp_ctx.__exit__(None, None, None)

    nc.compile()
    return nc


def _host_prep(hidden, target, lenseq, emb, w_ih, w_hh, b_ih, b_hh, w_out, b_out):
    assert not np.asarray(b_ih).any() and not np.asarray(b_hh).any() and not np.asarray(b_out).any(), (
        "nonzero biases not supported by this kernel build"
    )
    target = np.asarray(target)
    X = np.empty((SEQ, B), dtype=np.int64)
    X[0] = SOS_ID
    X[1:] = target[:SEQ - 1]
    done = ((X == EOS_ID) | (X == PAD_ID)).astype(np.float32)  # [SEQ, B]
    emb = np.asarray(emb, dtype=np.float32)
    E = emb[X.reshape(-1)]                                     # [2048, 512]
    eT = np.ascontiguousarray(E.T).astype(BF16)                # [512, 2048]
    h0 = np.asarray(hidden, dtype=np.float32)[0]               # [32, 1024]
    hT0 = np.ascontiguousarray(h0.T).astype(BF16)              # [1024, 32]
    done_s = np.ascontiguousarray(done.T)                      # [B, SEQ]
    done_row = done.reshape(ROWS)                              # [2048]
    done_pb = np.ascontiguousarray(done_row.reshape(MCH, 128).T)  # [128, 16]
    live_pb = np.ascontiguousarray(1.0 - done_pb)
    w_ih = np.asarray(w_ih, dtype=np.float32)
    w_hh = np.asarray(w_hh, dtype=np.float32)
    w_out = np.asarray(w_out, dtype=np.float32)

    in_maps = []
    for c in range(NC):
        rows = np.r_[c * HSL:(c + 1) * HSL,
                     HID + c * HSL:HID + (c + 1) * HSL,
                     2 * HID + c * HSL:2 * HID + (c + 1) * HSL]
        wihT = np.ascontiguousarray(w_ih[rows].T).astype(BF16)     # [512, 384]
        whhT = np.ascontiguousarray(w_hh[rows].T).astype(BF16)     # [1024, 384]
        woutT = np.ascontiguousarray(w_out[c * VS:(c + 1) * VS].T)  # [1024, 4000] f32
        # fp8 DoubleRow pair layout: wout8[p, u, j, n] = woutT[256u+128j+p, n]*WS
        w4 = (woutT * WS).reshape(HID // 256, 2, 128, VS)          # [u, j, p, n]
        wout8 = np.ascontiguousarray(w4.transpose(2, 0, 1, 3)).astype(FP8)  # [128,u,j,n]
        h0own = np.ascontiguousarray(h0[:, c * HSL:(c + 1) * HSL]).astype(BF16)
        pad_pb = done_pb if c == 0 else np.zeros_like(done_pb)
        in_maps.append({
            "eT": eT, "wihT": wihT, "whhT": whhT, "wout8": wout8,
            "hT0": hT0, "h0own": h0own, "doneS": done_s,
            "livePB": live_pb, "padPB": np.ascontiguousarray(pad_pb),
        })
    return in_maps


def kernel(hidden, target, lenseq, emb, w_ih, w_hh, b_ih, b_hh, w_out, b_out):
    from concourse.bass_utils import run_bass_kernel_spmd

    in_maps = _host_prep(hidden, target, lenseq, emb, w_ih, w_hh, b_ih, b_hh,
                         w_out, b_out)
    if "nc" not in _CACHE:
        _CACHE["nc"] = _build()
    res = run_bass_kernel_spmd(_CACHE["nc"], in_maps, core_ids=list(range(NC)))
    outs = [r["out"] for r in res.results]                     # each [2048, 4000]
    full = np.concatenate(outs, axis=1).reshape(SEQ, B, VOCAB)
    return full[:int(lenseq)]
